# revision 21
# baseline (speedup 1.0000x reference)
"""Bass/Trainium2 kernel for nn_Attention (B=4, N=2048, IN=256, HID=1024,
D=1024, OUT=256, H=8 heads), SPMD over 8 NeuronCores.

Sharding: core c handles batch b = c//2 and head-group g = c%2 (4 heads,
512 of the 1024 inner features).  Layer-1 of each QKV MLP is recomputed on
both cores of a batch (cheap); the output projection is computed per
head-group and the two partial products are summed on the host (plus bias).

Mask compaction: ~half the tokens are masked out (key mask) and masked
queries only output the bias row.  The host applies ONE permutation
(valid tokens first) to q, k and v inputs, so the kernel runs on
NP = ceil(max_valid/128)*128 tokens instead of N=2048.  Masked/padded key
rows get an additive -30000 before exp.

All DRAM inputs are pre-laid-out on the host as [128, X] SBUF images so
each loads with ONE contiguous DMA (DMA issue on the Sync engine costs
~0.6us each, so fusing dominates the kernel head).  Weights and x are
bf16; biases/mask constants f32.  The v-MLP bias bv2 is never added on
device: y = softmax(P)V + bv2 exactly, so bv2 @ Wp is folded into the
host-side output bias.

Phase B is head-interleaved: per key-tile kt the tensor queue gets
S(h0..h3) then y(h0..h3), so each y's Exp (Scalar engine) runs under the
other heads' S matmuls and the in-order tensor queue never stalls on the
S->exp->y chain.  Softmax denominators: bf16 accumulation of exp tiles
(vector), summed across partitions by an all-ones stationary matmul,
reciprocal_approx_fast straight off PSUM, one multiply per head (no
Ln/Exp recip, no act-table switch).

DMA: all in-flight transfers share aggregate HBM bandwidth round-robin
and completion has a ~4us latency floor, so the kernel head issues ONLY
the v-path bytes; the k-path / q-path loads are pinned behind phase-A
progress with 1-element WAW dependencies so their transfers start only
when due.  Tokens/queries beyond nv_max are never computed (the kT / v
tails are memset so the last key tile reads zeros).
"""

import numpy as np

B, N, IN_DIM, HID, D, OUT_DIM, H = 4, 2048, 256, 1024, 1024, 256, 8
NCORES = 8
HG = 2                 # head groups (cores per batch)
DG = D // HG           # 512 features per group
HEADS_G = H // HG      # 4 heads per core
Dh = D // H            # 128
NEG = -30000.0         # additive mask value (exp underflows to 0)

_CACHE = {}


def _chunks(total, size):
    out = []
    o = 0
    while o < total:
        s = min(size, total - o)
        out.append((o, s))
        o += s
    return out


def _build_nc(NP, QEND):
    import concourse.mybir as mybir
    import concourse.tile as tile
    from concourse import bacc
    from contextlib import ExitStack

    dt = mybir.dt
    f32 = dt.float32
    bf16 = dt.bfloat16
    AF = mybir.ActivationFunctionType
    ALU = mybir.AluOpType

    # Pin Exp to an activation-table set that also holds Tanh, so the
    # table-load pass never thrashes between sets.
    if not getattr(bacc, "_act_tables_patched", False):
        from concourse import hw_specs as _hw
        _orig_get = _hw.get_activation_tables

        def _patched(arch):
            tables = dict(_orig_get(arch))
            AFT = mybir.ActivationFunctionType
            keep = {"exp_and_others", "natural_log_exp_and_others"}
            for name in tables:
                if name in keep:
                    continue
                fns = tables[name]
                if AFT.Exp in fns or AFT.Ln in fns:
                    tables[name] = set()
            return tables

        _patched.__wrapped__ = _orig_get
        bacc.get_activation_tables = _patched
        bacc._act_tables_patched = True

    nc = bacc.Bacc("TRN2", target_bir_lowering=False, debug=False)

    NTOK = NP // 128             # 128-token key tiles
    KT1 = IN_DIM // 128          # 2  k-tiles in layer 1
    KT2 = HID // 128             # 8  k-tiles in layer 2
    MT1 = HID // 128             # 8  m-tiles in layer 1

    # tokens/queries beyond QEND = nv_max are dead: phase A computes
    # q/k/v only for [0, QEND) (the kT / v tails up to NP are memset to 0
    # so attention's last key tile reads zeros -> exp(-30000) -> 0), and
    # phase B emits queries only for [0, QEND).  Tail chunk LAST so its
    # epilogue is the only exposed one and it is tiny.
    TCH = _chunks(QEND, 512)
    QCH = _chunks(QEND, 512)

    NTC = len(TCH)

    # ---- DRAM I/O (everything already in its SBUF layout) ----
    # x images: [128, NTC, KT1, 512] bf16 (token-chunk major, so each
    # chunk is one CONTIGUOUS piece -- strided sub-2KB DMA lines run at
    # half speed)
    xs = {t: nc.dram_tensor(f"x{t}", [128, NTC * KT1 * 512], bf16,
                            kind="ExternalInput") for t in "vkq"}
    w1s = {t: nc.dram_tensor(f"w1{t}", [128, KT1 * HID], bf16,
                             kind="ExternalInput") for t in "vkq"}
    w2s = {t: nc.dram_tensor(f"w2{t}", [128, KT2 * DG], bf16,
                             kind="ExternalInput") for t in "vkq"}
    wpd = nc.dram_tensor("wpd", [128, HEADS_G * OUT_DIM], bf16,
                         kind="ExternalInput")
    # f32 constant pack: b1q|b1k|b1v (8 each) b2q|b2k (4 each) km (NTOK)
    # dneg (128)
    CC = 3 * MT1 + 2 * (DG // 128) + NTOK + 128
    cpk = nc.dram_tensor("cpk", [128, CC], f32, kind="ExternalInput")
    outY = nc.dram_tensor("outY", [128, 2 * NP], f32, kind="ExternalOutput")

    with tile.TileContext(nc) as tc, ExitStack() as ctx:
        # PSUM: two pools of 4 single-bank [128,512] f32 tiles
        psS = ctx.enter_context(tc.tile_pool(name="psS", bufs=4,
                                             space="PSUM"))
        psY = ctx.enter_context(tc.tile_pool(name="psY", bufs=4,
                                             space="PSUM"))
        singles = ctx.enter_context(tc.tile_pool(name="singles", bufs=1))
        h_pool = ctx.enter_context(tc.tile_pool(name="h", bufs=16))
        pt_pool = ctx.enter_context(tc.tile_pool(name="pt", bufs=8))
        sacc_pool = ctx.enter_context(tc.tile_pool(name="sacc", bufs=8))
        aux_pool = ctx.enter_context(tc.tile_pool(name="aux", bufs=5))
        ysc_pool = ctx.enter_context(tc.tile_pool(name="ysc", bufs=5))
        out_pool = ctx.enter_context(tc.tile_pool(name="out", bufs=2))

        # ---- input DMAs ----
        # All in-flight DMAs round-robin-share HBM bandwidth, so the
        # head only issues the v-path bytes (everything the first ~15us
        # of compute needs); the k-path and q-path issues are emitted on
        # the Scalar queue in the middle of phase A, which delays their
        # execution (in-order queue) until v's transfers have drained.
        x_sb = {}
        w1_sb = {}
        w2_sb = {}
        x_views = {}
        for t in "vkq":
            x_sb[t] = singles.tile([128, NTC, KT1, 512], bf16, tag=f"x{t}",
                                   name=f"x{t}")
            x_views[t] = xs[t].rearrange("p (c k n) -> p c k n",
                                         c=NTC, k=KT1)
            w1_sb[t] = singles.tile([128, MT1, KT1, 128], bf16,
                                    tag=f"w1{t}", name=f"w1{t}")
            w2_sb[t] = singles.tile([128, KT2, DG], bf16, tag=f"w2{t}",
                                    name=f"w2{t}")
        # group 1: ONLY what the very first matmuls need, so these few
        # transfers get the full aggregate bandwidth (DMA completion has
        # a ~4us fixed latency floor, so finer splits don't help)
        nc.sync.dma_start(out=x_sb["v"][:, 0], in_=x_views["v"][:, 0])
        nc.gpsimd.dma_start(out=w1_sb["v"], in_=w1s["v"][:, :])
        cpk_sb = singles.tile([128, CC], f32, tag="cpk")
        nc.gpsimd.dma_start(out=cpk_sb, in_=cpk[:, :])
        wp_sb = singles.tile([128, HEADS_G, OUT_DIM], bf16, tag="wp")

        # Later groups are pinned behind phase-A progress: a 1-element
        # copy of an h tile into the DMA's destination creates a WAW
        # dependency the scheduler must respect, so the transfer (and its
        # bandwidth share) starts only when its group is due.
        def _pin(dst, corner, src_view, trig):
            nc.gpsimd.tensor_copy(out=corner, in_=trig[0:1, 0:1])
            nc.sync.dma_start(out=dst, in_=src_view)

        def _dma_hooks(t, ci, m, trig):
            if t == "v" and ci == 0 and m == 0:
                if NTC > 1:
                    _pin(x_sb["v"][:, 1], x_sb["v"][0:1, 1, 0, 0:1],
                         x_views["v"][:, 1], trig)
                _pin(w2_sb["v"], w2_sb["v"][0:1, 0, 0:1],
                     w2s["v"][:, :], trig)
            if t == "v" and ci == 0 and m == MT1 - 1:
                for cj in range(2, NTC):
                    _pin(x_sb["v"][:, cj], x_sb["v"][0:1, cj, 0, 0:1],
                         x_views["v"][:, cj], trig)
            if t == "v" and ci == min(1, NTC - 1) and m == MT1 - 1:
                _pin(x_sb["k"], x_sb["k"][0:1, 0, 0, 0:1],
                     x_views["k"], trig)
                _pin(w1_sb["k"], w1_sb["k"][0:1, 0, 0, 0:1],
                     w1s["k"][:, :], trig)
                _pin(w2_sb["k"], w2_sb["k"][0:1, 0, 0:1],
                     w2s["k"][:, :], trig)
            if t == "k" and ci == 0 and m == MT1 - 1:
                _pin(x_sb["q"], x_sb["q"][0:1, 0, 0, 0:1],
                     x_views["q"], trig)
                _pin(w1_sb["q"], w1_sb["q"][0:1, 0, 0, 0:1],
                     w1s["q"][:, :], trig)
                _pin(w2_sb["q"], w2_sb["q"][0:1, 0, 0:1],
                     w2s["q"][:, :], trig)
                _pin(wp_sb, wp_sb[0:1, 0, 0:1], wpd[:, :], trig)

        # constant-pack slices
        o = 0
        b1 = {}
        for t in "qkv":
            b1[t] = cpk_sb[:, o:o + MT1]
            o += MT1
        b2 = {}
        for t in "qk":
            b2[t] = cpk_sb[:, o:o + DG // 128]
            o += DG // 128
        km = cpk_sb[:, o:o + NTOK]
        o += NTOK
        dneg = cpk_sb[:, o:o + 128]

        ones128 = singles.tile([128, 128], bf16, tag="ones128")
        nc.gpsimd.memset(ones128, 1.0)

        # persistent activations
        qT = [singles.tile([128, NP], bf16, tag=f"qT{i}", name=f"qT{i}")
              for i in range(HEADS_G)]
        kT = [singles.tile([128, NP], bf16, tag=f"kT{i}", name=f"kT{i}")
              for i in range(HEADS_G)]
        v_sb = [singles.tile([128, DG], bf16, tag=f"v{i}", name=f"v{i}")
                for i in range(NTOK)]
        if QEND < NP:
            for hd in range(HEADS_G):
                nc.gpsimd.memset(kT[hd][:, QEND:NP], 0.0)
            nc.gpsimd.memset(v_sb[NTOK - 1], 0.0)

        # ---------------- phase A: the three MLPs (v first) --------------
        for t in "vkq":
            for ci, (t0, tsz) in enumerate(TCH):
                tok_sl = slice(t0, t0 + tsz)
                # layer 1 (feature-major)
                h_sb = []
                for m in range(MT1):
                    p1 = psS.tile([128, 512], f32, tag="psS")
                    for k in range(KT1):
                        nc.tensor.matmul(
                            p1[:, :tsz],
                            w1_sb[t][:, m, k, :],
                            x_sb[t][:, ci, k, :tsz],
                            start=(k == 0), stop=(k == KT1 - 1),
                        )
                    ht = h_pool.tile([128, 512], bf16, tag="h")
                    nc.scalar.activation(
                        out=ht[:, :tsz], in_=p1[:, :tsz], func=AF.Tanh,
                        bias=b1[t][:, m:m + 1], scale=1.0,
                    )
                    h_sb.append(ht)
                    _dma_hooks(t, ci, m, ht)
                # layer 2
                if t in ("q", "k"):
                    dst = qT if t == "q" else kT
                    for m in range(DG // 128):       # head tiles
                        p2 = psY.tile([128, 512], f32, tag="psY")
                        for k in range(KT2):
                            nc.tensor.matmul(
                                p2[:, :tsz],
                                w2_sb[t][:, k, m * 128:(m + 1) * 128],
                                h_sb[k][:, :tsz],
                                start=(k == 0), stop=(k == KT2 - 1),
                            )
                        nc.vector.tensor_scalar_add(
                            out=dst[m][:, tok_sl], in0=p2[:, :tsz],
                            scalar1=b2[t][:, m:m + 1],
                        )
                else:
                    # v: token-major [tok, feat] tiles, one per 128 tokens
                    for tt in range((tsz + 127) // 128):
                        nt = min(128, tsz - tt * 128)
                        pv = psY.tile([128, 512], f32, tag="psY")
                        for k in range(KT2):
                            nc.tensor.matmul(
                                pv[:nt, :],
                                h_sb[k][:, tt * 128:tt * 128 + nt],
                                w2_sb[t][:, k, :],
                                start=(k == 0), stop=(k == KT2 - 1),
                            )
                        nc.vector.tensor_copy(
                            out=v_sb[t0 // 128 + tt][:nt, :],
                            in_=pv[:nt, :])

        # ---------------- phase B: attention + projection ----------------
        for q0, qw in QCH:
            y2 = [psY.tile([128, 512], f32, tag="psY", name=f"y2{hd}")
                  for hd in range(HEADS_G)]
            sacc = [sacc_pool.tile([128, 512], bf16, tag="sacc",
                                   name=f"sacc{hd}")
                    for hd in range(HEADS_G)]
            for kt in range(NTOK):
                pts = []
                for hd in range(HEADS_G):
                    st = psS.tile([128, 512], f32, tag="psS")
                    nc.tensor.matmul(
                        st[:, :qw],
                        kT[hd][:, kt * 128:(kt + 1) * 128],
                        qT[hd][:, q0:q0 + qw],
                        start=True, stop=True,
                    )
                    off = kt * 128 - q0
                    if 0 <= off < qw:
                        w = min(128, qw - off)
                        nc.vector.tensor_tensor(
                            st[:, off:off + w], st[:, off:off + w],
                            dneg[:, :w], ALU.add,
                        )
                    pt = pt_pool.tile([128, 512], bf16, tag="pt")
                    nc.scalar.activation(
                        out=pt[:, :qw], in_=st[:, :qw], func=AF.Exp,
                        bias=km[:, kt:kt + 1], scale=1.0,
                    )
                    eng = nc.vector
                    if kt == 0:
                        eng.tensor_copy(
                            out=sacc[hd][:, :qw], in_=pt[:, :qw])
                    else:
                        eng.tensor_tensor(
                            sacc[hd][:, :qw], sacc[hd][:, :qw],
                            pt[:, :qw], ALU.add)
                    pts.append(pt)
                for hd in range(HEADS_G):
                    nc.tensor.matmul(
                        y2[hd][:, :qw],
                        v_sb[kt][:, hd * 128:(hd + 1) * 128],
                        pts[hd][:, :qw],
                        start=(kt == 0), stop=(kt == NTOK - 1),
                    )
            # denominators: all-ones stationary matmul -> column sums
            # broadcast across partitions; then ONE DVE divide per head
            # (reads y2 straight from PSUM, freeing the bank).
            # epilogue: all aux sums first (tensor), then per-head
            # recip -> scale -> its projection contributions, head-major,
            # so proj(h) waits only on head h's chain, not on h3's.  The
            # output copies go to the Scalar engine, which is idle here
            # (the epilogue is otherwise vector-bound).
            auxs = []
            for hd in range(HEADS_G):
                aux = psS.tile([128, 512], f32, tag="psS")
                nc.tensor.matmul(
                    aux[:, :qw], ones128[:, :], sacc[hd][:, :qw],
                    start=True, stop=True,
                )
                auxs.append(aux)
            ot = out_pool.tile([128, 2, 512], f32, tag="out")
            pps = [psY.tile([128, 512], f32, tag="psY", name=f"pp{od}")
                   for od in range(OUT_DIM // 128)]
            for hd in range(HEADS_G):
                rb = aux_pool.tile([128, 512], f32, tag="aux", name="rb")
                nc.vector.reciprocal_approx_fast(
                    out=rb[:, :qw], in_=auxs[hd][:, :qw])
                ysc = ysc_pool.tile([128, 512], bf16, tag="ysc")
                nc.vector.tensor_tensor(
                    ysc[:, :qw], y2[hd][:, :qw], rb[:, :qw],
                    ALU.mult)
                for od in range(OUT_DIM // 128):
                    nc.tensor.matmul(
                        pps[od][:, :qw],
                        wp_sb[:, hd, od * 128:(od + 1) * 128],
                        ysc[:, :qw],
                        start=(hd == 0), stop=(hd == HEADS_G - 1),
                    )
            for od in range(OUT_DIM // 128):
                nc.scalar.copy(out=ot[:, od, :qw], in_=pps[od][:, :qw])
            nc.sync.dma_start(
                out=outY.rearrange("p (o n) -> p o n", o=2)[
                    :, :, q0:q0 + qw],
                in_=ot[:, :, :qw],
            )

    nc.compile()
    return nc


def _perm_np(mask_b):
    """Valid-first stable permutation and valid count for one batch."""
    maskf = mask_b.astype(np.float32)
    perm = np.argsort(1.0 - maskf, kind="stable")
    nv = int(maskf.sum())
    return perm, nv


def _img_km(x, kk):
    """[kk*128, M] -> SBUF image [128, kk*M] (k-tile major), contiguous."""
    m = x.shape[1]
    return np.ascontiguousarray(
        x.reshape(kk, 128, m).transpose(1, 0, 2).reshape(128, kk * m))


def _img_w1(w, kk):
    """[kk*128, M] -> [128, (M/128) * kk * 128] m-major image."""
    m = w.shape[1]
    return np.ascontiguousarray(
        w.reshape(kk, 128, m // 128, 128).transpose(1, 2, 0, 3)
        .reshape(128, m * kk))


def _img_x(xT, QEND):
    """x^T [IN, NP] -> [128, NTC*KT1*512] token-chunk-major image."""
    kk = xT.shape[0] // 128
    ntc = (QEND + 511) // 512
    out = np.zeros((128, ntc, kk, 512), np.float32)
    xr = xT.reshape(kk, 128, -1)
    for ci in range(ntc):
        t0 = ci * 512
        tsz = min(512, QEND - t0)
        out[:, ci, :, :tsz] = xr[:, :, t0:t0 + tsz].transpose(1, 0, 2)
    return out.reshape(128, ntc * kk * 512)


def _prep_core_inputs(inputs, b, g, NP, QEND=None):
    import ml_dtypes

    f32 = np.float32
    bf = ml_dtypes.bfloat16
    sl = slice(g * DG, (g + 1) * DG)
    scale = float(Dh) ** -0.5
    NTOK = NP // 128
    if QEND is None:
        QEND = int(inputs["mask"][:, :, 0].sum(axis=1).max())
    perm, nv = _perm_np(inputs["mask"][b, :, 0])
    km = np.full(NP, NEG, f32)
    km[:nv] = 0.0
    dn = np.zeros((128, 128), f32)
    np.fill_diagonal(dn, NEG)

    def ptok(x):   # permute tokens valid-first, pad to NP -> (NP, F)
        out = np.zeros((NP, x.shape[1]), f32)
        n = min(NP, x.shape[0])
        out[:n] = x[perm][:n]
        return out

    def ximg(x):   # (N, IN) -> token-chunk-major bf16 image of x^T
        return _img_x(np.ascontiguousarray(ptok(x).T), QEND).astype(bf)

    cpk = np.concatenate(
        [inputs["bq1"].astype(f32).reshape(HID // 128, 128).T,
         inputs["bk1"].astype(f32).reshape(HID // 128, 128).T,
         inputs["bv1"].astype(f32).reshape(HID // 128, 128).T,
         (inputs["bq2"][sl].astype(f32) * scale).reshape(DG // 128, 128).T,
         inputs["bk2"][sl].astype(f32).reshape(DG // 128, 128).T,
         km.reshape(NTOK, 128).T,
         dn],
        axis=1)

    return {
        "xq": ximg(inputs["query"][b]),
        "xk": ximg(inputs["key"][b]),
        "xv": ximg(inputs["value"][b]),
        "w1q": _img_w1(inputs["Wq1"].astype(f32), IN_DIM // 128).astype(bf),
        "w1k": _img_w1(inputs["Wk1"].astype(f32), IN_DIM // 128).astype(bf),
        "w1v": _img_w1(inputs["Wv1"].astype(f32), IN_DIM // 128).astype(bf),
        "w2q": _img_km(inputs["Wq2"][:, sl].astype(f32) * scale,
                       HID // 128).astype(bf),
        "w2k": _img_km(inputs["Wk2"][:, sl].astype(f32),
                       HID // 128).astype(bf),
        "w2v": _img_km(inputs["Wv2"][:, sl].astype(f32),
                       HID // 128).astype(bf),
        "wpd": _img_km(inputs["Wp"][sl, :].astype(f32),
                       DG // 128).astype(bf),
        "cpk": np.ascontiguousarray(cpk),
    }


def kernel(**inputs):
    import sys
    if "/opt/trn_rl_repo" not in sys.path:
        sys.path.insert(0, "/opt/trn_rl_repo")
    from concourse.bass_utils import run_bass_kernel_spmd

    inputs = {k: np.asarray(v) for k, v in inputs.items()}

    nv_max = int(inputs["mask"][:, :, 0].sum(axis=1).max())
    NP = ((nv_max + 127) // 128) * 128
    QEND = nv_max

    if _CACHE.get("key") != (NP, QEND):
        _CACHE["nc"] = _build_nc(NP, QEND)
        _CACHE["key"] = (NP, QEND)
        _CACHE["NP"] = NP
    nc = _CACHE["nc"]

    in_maps = [
        _prep_core_inputs(inputs, c // HG, c % HG, NP, QEND)
        for c in range(NCORES)
    ]

    res = run_bass_kernel_spmd(nc, in_maps, core_ids=list(range(NCORES)))
    results = res.results

    # bv2 is never added on device; softmax weights sum to 1, so its
    # contribution to the output is exactly bv2 @ Wp -- fold into the bias.
    bp = inputs["bp"].astype(np.float32) + (
        inputs["bv2"].astype(np.float32) @ inputs["Wp"].astype(np.float32))
    out = np.empty((B, N, OUT_DIM), np.float32)
    for b in range(B):
        acc = results[b * HG]["outY"].astype(np.float32)
        for g in range(1, HG):
            acc = acc + results[b * HG + g]["outY"].astype(np.float32)
        # [128, 2*NP] image -> (NP, 256)
        accT = acc.reshape(128, 2, NP).transpose(2, 1, 0).reshape(NP, 256)
        perm, nv = _perm_np(inputs["mask"][b, :, 0])
        out[b] = inputs["bp"].astype(np.float32)[None, :]
        out[b, perm[:nv]] = accT[:nv] + bp[None, :]
    return out
